# revision 1
# baseline (speedup 1.0000x reference)
"""Proven-working v1 (998us, rel err 3.5e-4): natural-layout scores,
ACT accum_out row sums, DMA-transpose of probabilities. Kept as fallback."""

import numpy as np
import ml_dtypes

from concourse import bacc, bass, tile, mybir
from concourse.bass_utils import run_bass_kernel_spmd

B, L, D = 16, 1024, 512
A = D
NCORES = 8
BLOC = B // NCORES
P = 128
DC = D // P
AC = A // P
LT = L // P
KC = L // P
NH = 512
SCALE = float(1.0 / np.sqrt(np.float32(D)))

F32 = mybir.dt.float32
BF16 = mybir.dt.bfloat16
EXP = mybir.ActivationFunctionType.Exp
COPY = mybir.ActivationFunctionType.Copy

W_NAMES = [f"{blk}_{w}" for blk in ("ta", "va", "tv")
           for w in ("kx", "qx", "vx", "ky", "qy", "vy")] + [
    "tav_k", "tav_q", "tav_v"]


def _build():
    nc = bacc.Bacc("TRN2", target_bir_lowering=False, debug=False,
                   num_devices=NCORES)

    mt_txt = nc.dram_tensor("mt_txt", (BLOC, D, L), BF16, kind="ExternalInput").ap()
    mt_au = nc.dram_tensor("mt_au", (BLOC, D, L), BF16, kind="ExternalInput").ap()
    mt_vi = nc.dram_tensor("mt_vi", (BLOC, D, L), BF16, kind="ExternalInput").ap()
    res = nc.dram_tensor("res", (3, BLOC, L, D), F32, kind="ExternalInput").ap()
    wt = nc.dram_tensor("wt", (21, D, A), BF16, kind="ExternalInput").ap()
    out = nc.dram_tensor("out", (BLOC, L, 4 * A), F32, kind="ExternalOutput").ap()

    with tile.TileContext(nc) as tc:
        _body(nc, tc, mt_txt, mt_au, mt_vi, res, wt, out)

    nc.compile()
    return nc


def _body(nc, tc, mt_txt, mt_au, mt_vi, res, wt, out):
    mt_dram = {"txt": mt_txt, "au": mt_au, "vi": mt_vi}

    with (
        tc.tile_pool(name="persist", bufs=1) as persist,
        tc.tile_pool(name="wpool", bufs=1) as wpool,
        tc.tile_pool(name="mpool", bufs=1) as mpool,
        tc.tile_pool(name="proj", bufs=1) as projp,
        tc.tile_pool(name="attn", bufs=2) as attnp,
        tc.tile_pool(name="small", bufs=3) as smallp,
        tc.tile_pool(name="ps_big", bufs=2, space=bass.MemorySpace.PSUM) as psb,
        tc.tile_pool(name="ps_small", bufs=4, space=bass.MemorySpace.PSUM) as pss,
    ):
        avT = [persist.tile([P, AC, L], BF16, tag=f"avT{b}", name=f"avT{b}")
               for b in range(BLOC)]

        def load_w(j):
            t = wpool.tile([P, DC, A], BF16, tag=f"w{j % 6}", name=f"w{j}")
            nc.sync.dma_start(out=t[:, :, :],
                              in_=wt[j].rearrange("(dc p) a -> p dc a", p=P))
            return t

        def load_mt(name, b, slot):
            t = mpool.tile([P, DC, L], BF16, tag=f"mT{slot}_{b}", name=f"mT_{name}{b}")
            nc.sync.dma_start(out=t[:, :, :],
                              in_=mt_dram[name][b].rearrange("(dc p) l -> p dc l", p=P))
            return t

        def proj_T(wtile, mtile, tag):
            o = projp.tile([P, AC, L], BF16, tag=tag, name=tag)
            for ac in range(AC):
                for h in range(L // NH):
                    ps = pss.tile([P, NH], F32, tag="ps_s", name="ps_pt")
                    for dc in range(DC):
                        nc.tensor.matmul(ps[:, :],
                                         wtile[:, dc, ac * P:(ac + 1) * P],
                                         mtile[:, dc, h * NH:(h + 1) * NH],
                                         start=(dc == 0), stop=(dc == DC - 1))
                    nc.vector.tensor_copy(o[:, ac, h * NH:(h + 1) * NH], ps[:, :])
            return o

        def proj_N(wtile, mtile, tag):
            o = projp.tile([P, KC, A], BF16, tag=tag, name=tag)
            for lt in range(LT):
                ps = pss.tile([P, NH], F32, tag="ps_s", name="ps_pn")
                for dc in range(DC):
                    nc.tensor.matmul(ps[:, :],
                                     mtile[:, dc, lt * P:(lt + 1) * P],
                                     wtile[:, dc, :],
                                     start=(dc == 0), stop=(dc == DC - 1))
                nc.vector.tensor_copy(o[:, lt, :], ps[:, :])
            return o

        def attention(qT, kT, v, writer):
            probsT = attnp.tile([P, KC, L], BF16, tag="probsT", name="probsT")
            sums = smallp.tile([P, LT], F32, tag="sums", name="sums")
            recip = smallp.tile([P, LT], F32, tag="recip", name="recip")
            for qt in range(LT):
                ps = psb.tile([P, L], F32, tag="scores", name="scores")
                for kh in range(L // NH):
                    for ac in range(AC):
                        nc.tensor.matmul(ps[:, kh * NH:(kh + 1) * NH],
                                         qT[:, ac, qt * P:(qt + 1) * P],
                                         kT[:, ac, kh * NH:(kh + 1) * NH],
                                         start=(ac == 0), stop=(ac == AC - 1))
                probs = attnp.tile([P, L], BF16, tag="probs", name="probs")
                nc.scalar.activation(probs[:, :], ps[:, :], EXP, scale=SCALE,
                                     accum_out=sums[:, qt:qt + 1])
                nc.scalar.dma_start_transpose(
                    out=probsT[:, :, qt * P:(qt + 1) * P], in_=probs[:, :])
                nc.vector.reciprocal(recip[:, qt:qt + 1], sums[:, qt:qt + 1])
            for qt in range(LT):
                po = pss.tile([P, A], F32, tag="ps_s", name="ps_pv")
                for kc in range(KC):
                    nc.tensor.matmul(po[:, :],
                                     probsT[:, kc, qt * P:(qt + 1) * P],
                                     v[:, kc, :],
                                     start=(kc == 0), stop=(kc == KC - 1))
                writer(qt, po, recip[:, qt:qt + 1])

        blocks = [(0, "txt", "au", 0), (1, "vi", "au", 2), (2, "txt", "vi", 1)]
        for blk, n1, n2, col in blocks:
            w = [load_w(blk * 6 + j) for j in range(6)]
            for b in range(BLOC):
                m1T = load_mt(n1, b, 1)
                m2T = load_mt(n2, b, 2)
                k1T = proj_T(w[0], m1T, "k1T")
                q2T = proj_T(w[4], m2T, "q2T")
                v1 = proj_N(w[2], m1T, "v1")
                k2T = proj_T(w[3], m2T, "k2T")
                q1T = proj_T(w[1], m1T, "q1T")
                v2 = proj_N(w[5], m2T, "v2")

                o1n = projp.tile([P, LT, A], BF16, tag="o1n", name="o1n")

                def writer1(qt, po, rc):
                    nc.scalar.activation(o1n[:, qt, :], po[:, :], COPY, scale=rc)

                def writer2(qt, po, rc, blk=blk, b=b, col=col):
                    o2n = smallp.tile([P, A], BF16, tag="o2n", name="o2n")
                    nc.scalar.activation(o2n[:, :], po[:, :], COPY, scale=rc)
                    res_t = smallp.tile([P, A], F32, tag="res_t", name="res_t")
                    nc.sync.dma_start(
                        out=res_t[:, :],
                        in_=res[blk, b, qt * P:(qt + 1) * P, :])
                    osum = smallp.tile([P, A], F32, tag="osum", name="osum")
                    nc.vector.tensor_add(osum[:, :], o1n[:, qt, :], o2n[:, :])
                    out_t = smallp.tile([P, A], F32, tag="out_t", name="out_t")
                    nc.vector.tensor_add(out_t[:, :], osum[:, :], res_t[:, :])
                    nc.sync.dma_start(
                        out=out[b, qt * P:(qt + 1) * P, col * A:(col + 1) * A],
                        in_=out_t[:, :])
                    if blk == 1:
                        av_bf = smallp.tile([P, A], BF16, tag="av_bf", name="av_bf")
                        nc.vector.tensor_copy(av_bf[:, :], out_t[:, :])
                        nc.scalar.dma_start_transpose(
                            out=avT[b][:, :, qt * P:(qt + 1) * P],
                            in_=av_bf[:, :])

                attention(q2T, k1T, v1, writer1)
                attention(q1T, k2T, v2, writer2)

        wk = load_w(18)
        wq = load_w(19)
        wv = load_w(20)
        for b in range(BLOC):
            xT = load_mt("txt", b, 1)
            kTc = proj_T(wk, xT, "k1T")
            qTc = proj_T(wq, avT[b], "q2T")
            vc = proj_N(wv, xT, "v1")

            def writer_c(qt, po, rc, b=b):
                out_t = smallp.tile([P, A], F32, tag="out_t", name="out_tc")
                nc.scalar.activation(out_t[:, :], po[:, :], COPY, scale=rc)
                nc.sync.dma_start(
                    out=out[b, qt * P:(qt + 1) * P, 3 * A:4 * A],
                    in_=out_t[:, :])

            attention(qTc, kTc, vc, writer_c)


_nc_cache = None
last_results = None


def _get_nc():
    global _nc_cache
    if _nc_cache is None:
        _nc_cache = _build()
    return _nc_cache


def kernel(**inputs):
    global last_results
    txt = np.asarray(inputs["txt"], dtype=np.float32)
    au = np.asarray(inputs["au"], dtype=np.float32)
    vi = np.asarray(inputs["vi"], dtype=np.float32)

    nat = {"txt": txt, "au": au, "vi": vi}
    mt = {n: np.ascontiguousarray(v.transpose(0, 2, 1)).astype(ml_dtypes.bfloat16)
          for n, v in nat.items()}
    wt_all = np.ascontiguousarray(
        np.stack([np.asarray(inputs[n], dtype=np.float32).T for n in W_NAMES])
    ).astype(ml_dtypes.bfloat16)
    res_all = np.stack([txt + au, vi + au, txt + vi])

    in_maps = []
    for c in range(NCORES):
        sl = slice(c * BLOC, (c + 1) * BLOC)
        in_maps.append({
            "mt_txt": mt["txt"][sl],
            "mt_au": mt["au"][sl],
            "mt_vi": mt["vi"][sl],
            "res": np.ascontiguousarray(res_all[:, sl]),
            "wt": wt_all,
        })

    nc = _get_nc()
    last_results = run_bass_kernel_spmd(nc, in_maps, core_ids=list(range(NCORES)))
    core_out = np.concatenate(
        [np.asarray(last_results.results[c]["out"]) for c in range(NCORES)], axis=0)
    return np.concatenate([txt, au, vi, core_out], axis=-1).astype(np.float32)



# revision 10
# speedup vs baseline: 1.4848x; 1.4848x over previous
"""v2: fp8 (e4m3) DoubleRow matmuls for the 6 symmetric attentions,
bf16 cross-attention; transposed scores (keys on partitions) so no
probability transposes; softmax sums via ones-matmul + tiny DRAM
roundtrip transpose. Sim-predicted rel err ~4.3e-3."""

import numpy as np
import ml_dtypes

from concourse import bacc, bass, tile, mybir
from concourse.bass_utils import run_bass_kernel_spmd

B, L, D = 16, 1024, 512
A = D
NCORES = 8
BLOC = B // NCORES
P = 128
DC = D // P
AC = A // P
LT = L // P
KC = L // P
NH = 512
SCALE = float(1.0 / np.sqrt(np.float32(D)))
EXP_BIAS = -1.0

F32 = mybir.dt.float32
BF16 = mybir.dt.bfloat16
F8 = mybir.dt.float8e4
DR = mybir.MatmulPerfMode.DoubleRow
EXP = mybir.ActivationFunctionType.Exp
COPY = mybir.ActivationFunctionType.Copy
MULT = mybir.AluOpType.mult
ADD = mybir.AluOpType.add

W_NAMES = [f"{blk}_{w}" for blk in ("ta", "va", "tv")
           for w in ("kx", "qx", "vx", "ky", "qy", "vy")] + [
    "tav_k", "tav_q", "tav_v"]


def _build():
    nc = bacc.Bacc("TRN2", target_bir_lowering=False, debug=False,
                   num_devices=NCORES)

    mt_txt = nc.dram_tensor("mt_txt", (BLOC, D, L), F8, kind="ExternalInput").ap()
    mt_au = nc.dram_tensor("mt_au", (BLOC, D, L), F8, kind="ExternalInput").ap()
    mt_vi = nc.dram_tensor("mt_vi", (BLOC, D, L), F8, kind="ExternalInput").ap()
    mtb_txt = nc.dram_tensor("mtb_txt", (BLOC, D, L), BF16, kind="ExternalInput").ap()
    res = nc.dram_tensor("res", (3, BLOC, L, D), F32, kind="ExternalInput").ap()
    wt8 = nc.dram_tensor("wt8", (18, D, A), F8, kind="ExternalInput").ap()
    wtb = nc.dram_tensor("wtb", (3, D, A), BF16, kind="ExternalInput").ap()
    out = nc.dram_tensor("out", (BLOC, L, 4 * A), F32, kind="ExternalOutput").ap()

    with tile.TileContext(nc) as tc:
        _body(nc, tc, mt_txt, mt_au, mt_vi, mtb_txt, res, wt8, wtb, out)

    nc.compile()
    return nc


def _body(nc, tc, mt_txt, mt_au, mt_vi, mtb_txt, res, wt8, wtb, out):
    mt_dram = {"txt": mt_txt, "au": mt_au, "vi": mt_vi}

    with (
        tc.tile_pool(name="persist", bufs=1) as persist,
        tc.tile_pool(name="wpool", bufs=1) as wpool,
        tc.tile_pool(name="mpool", bufs=1) as mpool,
        tc.tile_pool(name="proj", bufs=1) as projp,
        tc.tile_pool(name="attn", bufs=2) as attnp,
        tc.tile_pool(name="small", bufs=3) as smallp,
        tc.tile_pool(name="ps_a", bufs=2, space=bass.MemorySpace.PSUM) as psA,
        tc.tile_pool(name="ps_b", bufs=2, space=bass.MemorySpace.PSUM) as psB,
        tc.tile_pool(name="ps_s", bufs=1, space=bass.MemorySpace.PSUM) as psS,
    ):
        avT = [persist.tile([P, AC, L], BF16, tag=f"avT{b}", name=f"avT{b}")
               for b in range(BLOC)]
        # pad rows to 32B so the DoubleRow dual-fp8 ldweights stride is legal
        ones8 = persist.tile([P, KC, 32], F8, tag="ones8", name="ones8")
        nc.gpsimd.memset(ones8[:, :, :], 1.0)
        onesb = persist.tile([P, KC, 1], BF16, tag="onesb", name="onesb")
        nc.gpsimd.memset(onesb[:, :, :], 1.0)
        ebias = persist.tile([P, 1], F32, tag="ebias", name="ebias")
        nc.gpsimd.memset(ebias[:, :], EXP_BIAS)
        zbias = persist.tile([P, 1], F32, tag="zbias", name="zbias")
        nc.gpsimd.memset(zbias[:, :], 0.0)
        ident = persist.tile([1, 1], F32, tag="ident", name="ident")
        nc.gpsimd.memset(ident[:, :], 1.0)

        # round-robin the PSUM->SBUF projection copies across DVE / Scalar
        # (GPSIMD cannot access PSUM)
        _ctr = [0]

        def copy_eng():
            _ctr[0] += 1
            if _ctr[0] % 3 == 0:
                return nc.scalar.copy
            return nc.vector.tensor_copy

        def load_w8(j):
            t = wpool.tile([P, DC, A], F8, tag=f"w{j % 6}", name=f"w{j}")
            nc.sync.dma_start(out=t[:, :, :],
                              in_=wt8[j].rearrange("(dc p) a -> p dc a", p=P))
            return t

        def load_wb(j):
            t = wpool.tile([P, DC, A], BF16, tag=f"wb{j}", name=f"wb{j}")
            nc.sync.dma_start(out=t[:, :, :],
                              in_=wtb[j].rearrange("(dc p) a -> p dc a", p=P))
            return t

        def load_mt8(name, b, slot):
            t = mpool.tile([P, DC, L], F8, tag=f"mT{slot}_{b}", name=f"mT_{name}{b}")
            nc.sync.dma_start(out=t[:, :, :],
                              in_=mt_dram[name][b].rearrange("(dc p) l -> p dc l", p=P))
            return t

        def load_mtb(b):
            t = mpool.tile([P, DC, L], BF16, tag=f"mTc_{b}", name=f"mTc_txt{b}")
            nc.sync.dma_start(out=t[:, :, :],
                              in_=mtb_txt[b].rearrange("(dc p) l -> p dc l", p=P))
            return t

        def proj_T(wtile, mtile, tag, fp8):
            o = projp.tile([P, AC, L], F8 if fp8 else BF16, tag=tag, name=tag)
            for ac in range(AC):
                ps = psA.tile([P, 2, NH], F32, tag="psA", name="ps_pt")
                for h in range(2):
                    if fp8:
                        for dc in (0, 2):
                            nc.tensor.matmul(
                                ps[:, h, :],
                                wtile[:, dc:dc + 2, ac * P:(ac + 1) * P],
                                mtile[:, dc:dc + 2, h * NH:(h + 1) * NH],
                                start=(dc == 0), stop=(dc == 2), perf_mode=DR)
                    else:
                        for dc in range(DC):
                            nc.tensor.matmul(
                                ps[:, h, :],
                                wtile[:, dc, ac * P:(ac + 1) * P],
                                mtile[:, dc, h * NH:(h + 1) * NH],
                                start=(dc == 0), stop=(dc == DC - 1))
                copy_eng()(
                    o[:, ac, :].rearrange("p (h x) -> p h x", h=2), ps[:, :, :])
            return o

        def proj_N(wtile, mtile, tag, fp8):
            o = projp.tile([P, KC, A], F8 if fp8 else BF16, tag=tag, name=tag)
            for lt2 in range(0, LT, 2):
                ps = psA.tile([P, 2, A], F32, tag="psA", name="ps_pn")
                for j in range(2):
                    lt = lt2 + j
                    if fp8:
                        for dc in (0, 2):
                            nc.tensor.matmul(
                                ps[:, j, :],
                                mtile[:, dc:dc + 2, lt * P:(lt + 1) * P],
                                wtile[:, dc:dc + 2, :],
                                start=(dc == 0), stop=(dc == 2), perf_mode=DR)
                    else:
                        for dc in range(DC):
                            nc.tensor.matmul(
                                ps[:, j, :],
                                mtile[:, dc, lt * P:(lt + 1) * P],
                                wtile[:, dc, :],
                                start=(dc == 0), stop=(dc == DC - 1))
                copy_eng()(o[:, lt2:lt2 + 2, :], ps[:, :, :])
            return o

        def attention(qT, kT, v, writer, fp8):
            pdt = F8 if fp8 else BF16
            ones = ones8 if fp8 else onesb
            probsT = attnp.tile([P, KC, L], pdt,
                                tag="probsT8" if fp8 else "probsTb", name="probsT")
            # scores transposed: keys on partitions, queries along free dim
            for kt in range(KC):
                for qh in range(2):
                    ps = psB.tile([P, NH], F32, tag="scB", name="scores")
                    if fp8:
                        for ac in (0, 2):
                            nc.tensor.matmul(
                                ps[:, :],
                                kT[:, ac:ac + 2, kt * P:(kt + 1) * P],
                                qT[:, ac:ac + 2, qh * NH:(qh + 1) * NH],
                                start=(ac == 0), stop=(ac == 2), perf_mode=DR)
                    else:
                        for ac in range(AC):
                            nc.tensor.matmul(
                                ps[:, :],
                                kT[:, ac, kt * P:(kt + 1) * P],
                                qT[:, ac, qh * NH:(qh + 1) * NH],
                                start=(ac == 0), stop=(ac == AC - 1))
                    nc.scalar.activation(
                        probsT[:, kt, qh * NH:(qh + 1) * NH], ps[:, :], EXP,
                        scale=SCALE, bias=(ebias[:, 0:1] if fp8 else zbias[:, 0:1]))
            # key-sums per query via ones-matmul -> [1, L] PSUM
            sumsP = psS.tile([1, L], F32, tag="sums", name="sums")
            for qh in range(2):
                if fp8:
                    for ktp in (0, 2, 4, 6):
                        nc.tensor.matmul(
                            sumsP[0:1, qh * NH:(qh + 1) * NH],
                            ones[:, ktp:ktp + 2, 0:1],
                            probsT[:, ktp:ktp + 2, qh * NH:(qh + 1) * NH],
                            start=(ktp == 0), stop=(ktp == 6), perf_mode=DR)
                else:
                    for kt in range(KC):
                        nc.tensor.matmul(
                            sumsP[0:1, qh * NH:(qh + 1) * NH],
                            ones[:, kt, 0:1],
                            probsT[:, kt, qh * NH:(qh + 1) * NH],
                            start=(kt == 0), stop=(kt == KC - 1))
            sums_sb = smallp.tile([1, L], F32, tag="sums_sb", name="sums_sb", bufs=2)
            nc.scalar.copy(sums_sb[0:1, :], sumsP[0:1, :])

            def pv_pair(qt2):
                po = psA.tile([P, 2, A], F32, tag="psA", name="ps_pv")
                for j in range(2):
                    qt = qt2 + j
                    if fp8:
                        for kc in (0, 2, 4, 6):
                            nc.tensor.matmul(
                                po[:, j, :],
                                probsT[:, kc:kc + 2, qt * P:(qt + 1) * P],
                                v[:, kc:kc + 2, :],
                                start=(kc == 0), stop=(kc == 6), perf_mode=DR)
                    else:
                        for kc in range(KC):
                            nc.tensor.matmul(
                                po[:, j, :],
                                probsT[:, kc, qt * P:(qt + 1) * P],
                                v[:, kc, :],
                                start=(kc == 0), stop=(kc == KC - 1))
                return po

            # first PV pair keeps the PE busy while scalar copies the sums out
            po0 = pv_pair(0)
            # transpose sums [1, L] -> [P, LT] on the PE (8 tiny transposes
            # into one PSUM bank; only the first carries start=True so the
            # bank zero-fill happens once)
            sumsT = psB.tile([P, NH], F32, tag="scB", name="sumsT")
            for qt in range(LT):
                nc.tensor.matmul(
                    sumsT[:, qt:qt + 1], sums_sb[0:1, qt * P:(qt + 1) * P],
                    ident[0:1, 0:1], start=(qt == 0), stop=(qt == LT - 1),
                    is_transpose=True, skip_group_check=True)
            rcT = smallp.tile([P, LT], F32, tag="rcT", name="rcT")
            nc.vector.reciprocal(rcT[:, :], sumsT[:, 0:LT])
            for j in range(2):
                writer(j, po0[:, j, :], rcT[:, j:j + 1])
            for qt2 in range(2, LT, 2):
                po = pv_pair(qt2)
                for j in range(2):
                    qt = qt2 + j
                    writer(qt, po[:, j, :], rcT[:, qt:qt + 1])

        blocks = [(0, "txt", "au", 0), (1, "vi", "au", 2), (2, "txt", "vi", 1)]
        for blk, n1, n2, col in blocks:
            w = [load_w8(blk * 6 + j) for j in range(6)]
            for b in range(BLOC):
                m1T = load_mt8(n1, b, 1)
                m2T = load_mt8(n2, b, 2)
                k1T = proj_T(w[0], m1T, "k1T", True)
                q2T = proj_T(w[4], m2T, "q2T", True)
                v1 = proj_N(w[2], m1T, "v1", True)
                k2T = proj_T(w[3], m2T, "k2T", True)
                q1T = proj_T(w[1], m1T, "q1T", True)
                v2 = proj_N(w[5], m2T, "v2", True)

                o1r = projp.tile([P, LT, A], F32, tag="o1r", name="o1r")

                def writer1(qt, po, rc, blk=blk, b=b):
                    res_t = smallp.tile([P, A], F32, tag="res_t", name="res_t")
                    nc.sync.dma_start(
                        out=res_t[:, :],
                        in_=res[blk, b, qt * P:(qt + 1) * P, :])
                    nc.vector.scalar_tensor_tensor(
                        o1r[:, qt, :], po, rc, res_t[:, :], op0=MULT, op1=ADD)

                def writer2(qt, po, rc, blk=blk, b=b, col=col):
                    out_t = smallp.tile([P, A], F32, tag="out_t", name="out_t")
                    nc.vector.scalar_tensor_tensor(
                        out_t[:, :], po, rc, o1r[:, qt, :], op0=MULT, op1=ADD)
                    nc.sync.dma_start(
                        out=out[b, qt * P:(qt + 1) * P, col * A:(col + 1) * A],
                        in_=out_t[:, :])
                    if blk == 1:
                        av_bf = smallp.tile([P, A], BF16, tag="av_bf", name="av_bf")
                        nc.gpsimd.tensor_copy(av_bf[:, :], out_t[:, :])
                        nc.scalar.dma_start_transpose(
                            out=avT[b][:, :, qt * P:(qt + 1) * P],
                            in_=av_bf[:, :])

                attention(q2T, k1T, v1, writer1, True)
                attention(q1T, k2T, v2, writer2, True)

        # cross attention (bf16): x=txt, queries=av
        wk = load_wb(0)
        wq = load_wb(1)
        wv = load_wb(2)
        for b in range(BLOC):
            xT = load_mtb(b)
            kTc = proj_T(wk, xT, "kTc", False)
            qTc = proj_T(wq, avT[b], "qTc", False)
            vc = proj_N(wv, xT, "vc", False)

            def writer_c(qt, po, rc, b=b):
                out_t = smallp.tile([P, A], F32, tag="out_t", name="out_tc")
                nc.scalar.activation(out_t[:, :], po, COPY, scale=rc)
                nc.sync.dma_start(
                    out=out[b, qt * P:(qt + 1) * P, 3 * A:4 * A],
                    in_=out_t[:, :])

            attention(qTc, kTc, vc, writer_c, False)


_nc_cache = None
last_results = None


def _get_nc():
    global _nc_cache
    if _nc_cache is None:
        _nc_cache = _build()
    return _nc_cache


def kernel(**inputs):
    global last_results
    txt = np.asarray(inputs["txt"], dtype=np.float32)
    au = np.asarray(inputs["au"], dtype=np.float32)
    vi = np.asarray(inputs["vi"], dtype=np.float32)

    nat = {"txt": txt, "au": au, "vi": vi}
    mtT = {n: np.ascontiguousarray(v.transpose(0, 2, 1)) for n, v in nat.items()}
    mt8 = {n: v.astype(ml_dtypes.float8_e4m3) for n, v in mtT.items()}
    mtb = mtT["txt"].astype(ml_dtypes.bfloat16)
    wT = [np.ascontiguousarray(np.asarray(inputs[n], dtype=np.float32).T)
          for n in W_NAMES]
    wt8_all = np.stack(wT[:18]).astype(ml_dtypes.float8_e4m3)
    wtb_all = np.stack(wT[18:]).astype(ml_dtypes.bfloat16)
    res_all = np.stack([txt + au, vi + au, txt + vi])

    in_maps = []
    for c in range(NCORES):
        sl = slice(c * BLOC, (c + 1) * BLOC)
        in_maps.append({
            "mt_txt": mt8["txt"][sl],
            "mt_au": mt8["au"][sl],
            "mt_vi": mt8["vi"][sl],
            "mtb_txt": mtb[sl],
            "res": np.ascontiguousarray(res_all[:, sl]),
            "wt8": wt8_all,
            "wtb": wtb_all,
        })

    nc = _get_nc()
    last_results = run_bass_kernel_spmd(nc, in_maps, core_ids=list(range(NCORES)))
    core_out = np.concatenate(
        [np.asarray(last_results.results[c]["out"]) for c in range(NCORES)], axis=0)
    return np.concatenate([txt, au, vi, core_out], axis=-1).astype(np.float32)


# revision 12
# speedup vs baseline: 1.5811x; 1.0648x over previous
"""v3: all-fp8 (e4m3) DoubleRow matmuls for all 7 attentions; transposed
scores (keys on partitions, no probability transposes); softmax sums via
ones-matmul into PSUM + PE transpose of the [1,L] sums row; interleaved
score/PV phases of the two symmetric attentions to hide exp latency.
Sim-predicted rel err ~7.6e-3 (gate 2e-2)."""

import numpy as np
import ml_dtypes

from concourse import bacc, bass, tile, mybir
from concourse.bass_utils import run_bass_kernel_spmd

B, L, D = 16, 1024, 512
A = D
NCORES = 8
BLOC = B // NCORES
P = 128
DC = D // P
AC = A // P
LT = L // P
KC = L // P
NH = 512
SCALE = float(1.0 / np.sqrt(np.float32(D)))
EXP_BIAS = -1.0   # symmetric attns: keeps exp() under fp8e4 max 240
CROSS_BIAS = -5.0  # cross attn has wider score range (queries = av)

F32 = mybir.dt.float32
BF16 = mybir.dt.bfloat16
F8 = mybir.dt.float8e4
DR = mybir.MatmulPerfMode.DoubleRow
EXP = mybir.ActivationFunctionType.Exp
COPY = mybir.ActivationFunctionType.Copy
MULT = mybir.AluOpType.mult
ADD = mybir.AluOpType.add

W_NAMES = [f"{blk}_{w}" for blk in ("ta", "va", "tv")
           for w in ("kx", "qx", "vx", "ky", "qy", "vy")] + [
    "tav_k", "tav_q", "tav_v"]


def _build():
    nc = bacc.Bacc("TRN2", target_bir_lowering=False, debug=False,
                   num_devices=NCORES)

    mt_txt = nc.dram_tensor("mt_txt", (BLOC, D, L), F8, kind="ExternalInput").ap()
    mt_au = nc.dram_tensor("mt_au", (BLOC, D, L), F8, kind="ExternalInput").ap()
    mt_vi = nc.dram_tensor("mt_vi", (BLOC, D, L), F8, kind="ExternalInput").ap()
    res = nc.dram_tensor("res", (3, BLOC, L, D), F32, kind="ExternalInput").ap()
    wt8 = nc.dram_tensor("wt8", (21, D, A), F8, kind="ExternalInput").ap()
    out = nc.dram_tensor("out", (BLOC, L, 4 * A), F32, kind="ExternalOutput").ap()

    with tile.TileContext(nc) as tc:
        _body(nc, tc, mt_txt, mt_au, mt_vi, res, wt8, out)

    nc.compile()
    return nc


def _body(nc, tc, mt_txt, mt_au, mt_vi, res, wt8, out):
    mt_dram = {"txt": mt_txt, "au": mt_au, "vi": mt_vi}

    with (
        tc.tile_pool(name="persist", bufs=1) as persist,
        tc.tile_pool(name="wpool", bufs=1) as wpool,
        tc.tile_pool(name="mpool", bufs=1) as mpool,
        tc.tile_pool(name="proj", bufs=1) as projp,
        tc.tile_pool(name="attn", bufs=2) as attnp,
        tc.tile_pool(name="small", bufs=3) as smallp,
        tc.tile_pool(name="ps_a", bufs=3, space=bass.MemorySpace.PSUM) as psA,
        tc.tile_pool(name="ps_b", bufs=2, space=bass.MemorySpace.PSUM) as psB,
    ):
        avT = [persist.tile([P, AC, L], BF16, tag=f"avT{b}", name=f"avT{b}")
               for b in range(BLOC)]
        avT8 = [persist.tile([P, AC, L], F8, tag=f"avT8{b}", name=f"avT8{b}")
                for b in range(BLOC)]
        # pad rows to 32B so the DoubleRow dual-fp8 ldweights stride is legal
        ones8 = persist.tile([P, KC, 32], F8, tag="ones8", name="ones8")
        nc.gpsimd.memset(ones8[:, :, :], 1.0)
        ebias = persist.tile([P, 1], F32, tag="ebias", name="ebias")
        nc.gpsimd.memset(ebias[:, :], EXP_BIAS)
        cbias = persist.tile([P, 1], F32, tag="cbias", name="cbias")
        nc.gpsimd.memset(cbias[:, :], CROSS_BIAS)
        ident = persist.tile([1, 1], F32, tag="ident", name="ident")
        nc.gpsimd.memset(ident[:, :], 1.0)

        # round-robin the PSUM->SBUF projection copies across DVE / Scalar
        # (GPSIMD cannot access PSUM)
        _ctr = [0]

        def copy_eng():
            _ctr[0] += 1
            if _ctr[0] % 3 == 0:
                return nc.scalar.copy
            return nc.vector.tensor_copy

        def load_w8(j, slot):
            t = wpool.tile([P, DC, A], F8, tag=f"w{slot}", name=f"w{j}")
            nc.sync.dma_start(out=t[:, :, :],
                              in_=wt8[j].rearrange("(dc p) a -> p dc a", p=P))
            return t

        def load_mt8(name, b, slot):
            t = mpool.tile([P, DC, L], F8, tag=f"mT{slot}_{b}", name=f"mT_{name}{b}")
            nc.sync.dma_start(out=t[:, :, :],
                              in_=mt_dram[name][b].rearrange("(dc p) l -> p dc l", p=P))
            return t

        def proj_T(wtile, mtile, tag):
            o = projp.tile([P, AC, L], F8, tag=tag, name=tag)
            for ac in range(AC):
                ps = psA.tile([P, 2, NH], F32, tag="psA", name="ps_pt")
                for h in range(2):
                    for dc in (0, 2):
                        nc.tensor.matmul(
                            ps[:, h, :],
                            wtile[:, dc:dc + 2, ac * P:(ac + 1) * P],
                            mtile[:, dc:dc + 2, h * NH:(h + 1) * NH],
                            start=(dc == 0), stop=(dc == 2), perf_mode=DR)
                copy_eng()(
                    o[:, ac, :].rearrange("p (h x) -> p h x", h=2), ps[:, :, :])
            return o

        def proj_N(wtile, mtile, tag):
            o = projp.tile([P, KC, A], F8, tag=tag, name=tag)
            for lt2 in range(0, LT, 2):
                ps = psA.tile([P, 2, A], F32, tag="psA", name="ps_pn")
                for j in range(2):
                    lt = lt2 + j
                    for dc in (0, 2):
                        nc.tensor.matmul(
                            ps[:, j, :],
                            mtile[:, dc:dc + 2, lt * P:(lt + 1) * P],
                            wtile[:, dc:dc + 2, :],
                            start=(dc == 0), stop=(dc == 2), perf_mode=DR)
                copy_eng()(o[:, lt2:lt2 + 2, :], ps[:, :, :])
            return o

        def score_phase(qT, kT, bias):
            """scores (keys on partitions) -> exp -> probsT fp8; key-sums
            via ones-matmul; returns (probsT, sums_sb [1, L] f32 in SBUF)."""
            probsT = attnp.tile([P, KC, L], F8, tag="probsT8", name="probsT")
            for kt in range(KC):
                for qh in range(2):
                    ps = psB.tile([P, NH], F32, tag="scB", name="scores")
                    for ac in (0, 2):
                        nc.tensor.matmul(
                            ps[:, :],
                            kT[:, ac:ac + 2, kt * P:(kt + 1) * P],
                            qT[:, ac:ac + 2, qh * NH:(qh + 1) * NH],
                            start=(ac == 0), stop=(ac == 2), perf_mode=DR)
                    nc.scalar.activation(
                        probsT[:, kt, qh * NH:(qh + 1) * NH], ps[:, :], EXP,
                        scale=SCALE, bias=bias)
            sums_sb = smallp.tile([1, L], F32, tag="sums_sb", name="sums_sb",
                                  bufs=2)
            for qh in range(2):
                st = psB.tile([P, NH], F32, tag="scB", name="sums")
                for ktp in (0, 2, 4, 6):
                    nc.tensor.matmul(
                        st[0:1, :],
                        ones8[:, ktp:ktp + 2, 0:1],
                        probsT[:, ktp:ktp + 2, qh * NH:(qh + 1) * NH],
                        start=(ktp == 0), stop=(ktp == 6), perf_mode=DR)
                nc.scalar.copy(sums_sb[0:1, qh * NH:(qh + 1) * NH], st[0:1, :])
            return probsT, sums_sb

        def pv_phase(probsT, sums_sb, v, writer):
            def pv_pair(qt2):
                po = psA.tile([P, 2, A], F32, tag="psA", name="ps_pv")
                for j in range(2):
                    qt = qt2 + j
                    for kc in (0, 2, 4, 6):
                        nc.tensor.matmul(
                            po[:, j, :],
                            probsT[:, kc:kc + 2, qt * P:(qt + 1) * P],
                            v[:, kc:kc + 2, :],
                            start=(kc == 0), stop=(kc == 6), perf_mode=DR)
                return po

            # first PV pair keeps the PE busy while scalar copies the sums out
            po0 = pv_pair(0)
            # transpose sums [1, L] -> [P, LT] on the PE (8 tiny transposes
            # into one PSUM bank; only the first carries start=True so the
            # bank zero-fill happens once)
            sumsT = psB.tile([P, NH], F32, tag="scB", name="sumsT")
            for qt in range(LT):
                nc.tensor.matmul(
                    sumsT[:, qt:qt + 1], sums_sb[0:1, qt * P:(qt + 1) * P],
                    ident[0:1, 0:1], start=(qt == 0), stop=(qt == LT - 1),
                    is_transpose=True, skip_group_check=True)
            rcT = smallp.tile([P, LT], F32, tag="rcT", name="rcT")
            nc.vector.reciprocal(rcT[:, :], sumsT[:, 0:LT])
            for j in range(2):
                writer(j, po0[:, j, :], rcT[:, j:j + 1])
            for qt2 in range(2, LT, 2):
                po = pv_pair(qt2)
                for j in range(2):
                    qt = qt2 + j
                    writer(qt, po[:, j, :], rcT[:, qt:qt + 1])

        blocks = [(0, "txt", "au", 0), (1, "vi", "au", 2), (2, "txt", "vi", 1)]
        for blk, n1, n2, col in blocks:
            w = [load_w8(blk * 6 + j, f"{blk % 2}_{j}") for j in range(6)]
            for b in range(BLOC):
                m1T = load_mt8(n1, b, 1)
                m2T = load_mt8(n2, b, 2)
                k1T = proj_T(w[0], m1T, "k1T")
                q2T = proj_T(w[4], m2T, "q2T")
                v1 = proj_N(w[2], m1T, "v1")
                k2T = proj_T(w[3], m2T, "k2T")
                q1T = proj_T(w[1], m1T, "q1T")
                v2 = proj_N(w[5], m2T, "v2")

                o1r = projp.tile([P, LT, A], F32, tag="o1r", name="o1r")

                def writer1(qt, po, rc, blk=blk, b=b):
                    res_t = smallp.tile([P, A], F32, tag="res_t", name="res_t")
                    nc.sync.dma_start(
                        out=res_t[:, :],
                        in_=res[blk, b, qt * P:(qt + 1) * P, :])
                    nc.vector.scalar_tensor_tensor(
                        o1r[:, qt, :], po, rc, res_t[:, :], op0=MULT, op1=ADD)

                def writer2(qt, po, rc, blk=blk, b=b, col=col):
                    out_t = smallp.tile([P, A], F32, tag="out_t", name="out_t")
                    nc.vector.scalar_tensor_tensor(
                        out_t[:, :], po, rc, o1r[:, qt, :], op0=MULT, op1=ADD)
                    nc.sync.dma_start(
                        out=out[b, qt * P:(qt + 1) * P, col * A:(col + 1) * A],
                        in_=out_t[:, :])
                    if blk == 1:
                        av_bf = smallp.tile([P, A], BF16, tag="av_bf", name="av_bf")
                        nc.gpsimd.tensor_copy(av_bf[:, :], out_t[:, :])
                        nc.scalar.dma_start_transpose(
                            out=avT[b][:, :, qt * P:(qt + 1) * P],
                            in_=av_bf[:, :])

                p1, s1 = score_phase(q2T, k1T, ebias[:, 0:1])
                p2, s2 = score_phase(q1T, k2T, ebias[:, 0:1])
                pv_phase(p1, s1, v1, writer1)
                pv_phase(p2, s2, v2, writer2)

        # cross attention (fp8): x=txt, queries=av
        wk = load_w8(18, "0_0")
        wq = load_w8(19, "0_1")
        wv = load_w8(20, "0_2")
        for b in range(BLOC):
            nc.vector.tensor_copy(avT8[b][:, :, :], avT[b][:, :, :])
            xT = load_mt8("txt", b, 1)
            kTc = proj_T(wk, xT, "kTc")
            qTc = proj_T(wq, avT8[b], "qTc")
            vc = proj_N(wv, xT, "vc")

            def writer_c(qt, po, rc, b=b):
                out_t = smallp.tile([P, A], F32, tag="out_t", name="out_tc")
                nc.scalar.activation(out_t[:, :], po, COPY, scale=rc)
                nc.sync.dma_start(
                    out=out[b, qt * P:(qt + 1) * P, 3 * A:4 * A],
                    in_=out_t[:, :])

            pc, sc = score_phase(qTc, kTc, cbias[:, 0:1])
            pv_phase(pc, sc, vc, writer_c)


_nc_cache = None
last_results = None


def _get_nc():
    global _nc_cache
    if _nc_cache is None:
        _nc_cache = _build()
    return _nc_cache


def kernel(**inputs):
    global last_results
    txt = np.asarray(inputs["txt"], dtype=np.float32)
    au = np.asarray(inputs["au"], dtype=np.float32)
    vi = np.asarray(inputs["vi"], dtype=np.float32)

    nat = {"txt": txt, "au": au, "vi": vi}
    mt8 = {n: np.ascontiguousarray(v.transpose(0, 2, 1)).astype(ml_dtypes.float8_e4m3)
           for n, v in nat.items()}
    wt8_all = np.ascontiguousarray(
        np.stack([np.asarray(inputs[n], dtype=np.float32).T for n in W_NAMES])
    ).astype(ml_dtypes.float8_e4m3)
    res_all = np.stack([txt + au, vi + au, txt + vi])

    in_maps = []
    for c in range(NCORES):
        sl = slice(c * BLOC, (c + 1) * BLOC)
        in_maps.append({
            "mt_txt": mt8["txt"][sl],
            "mt_au": mt8["au"][sl],
            "mt_vi": mt8["vi"][sl],
            "res": np.ascontiguousarray(res_all[:, sl]),
            "wt8": wt8_all,
        })

    nc = _get_nc()
    last_results = run_bass_kernel_spmd(nc, in_maps, core_ids=list(range(NCORES)))
    core_out = np.concatenate(
        [np.asarray(last_results.results[c]["out"]) for c in range(NCORES)], axis=0)
    return np.concatenate([txt, au, vi, core_out], axis=-1).astype(np.float32)
